# revision 5
# baseline (speedup 1.0000x reference)
"""JKNet (3x GraphConv+LN+ReLU, JK-concat, Linear, LN) on 8 Trainium2 cores.

v3 strategy:
- Nodes dealt round-robin (by global in-degree rank) to 8 cores; within a
  core, nodes are ordered by (-lo_cnt, -hi_cnt) into 49 blocks of 128
  (partition = node). lo/hi = whether an edge's source lives in cores 0-4
  (table rows [0, 31250)) or cores 5-7 ([31250, 50000)) — both halves fit
  int16 indices for the batched `dma_gather` (Ant) instruction.
- Per block, node v's lo-edges occupy slot columns [0, Klo_b), hi-edges
  [Klo_b, Klo_b+Khi_b), padded to block maxima (shared across cores for an
  SPMD-uniform program). Slot (column, partition) indices are gathered with
  one dma_gather per (batch of blocks, half) — ~1us fixed cost amortized
  over thousands of rows (vs 1us per 128 rows for indirect_dma_start).
- Per-edge factors q = ew * rsqrt(deg_out[src]) * rsqrt(deg_in[dst]) are
  applied by one wide in-place DVE multiply per (block, half) against an
  SBUF-resident replicated Q table (first few blocks fall back to per-column
  tensor_scalar to fit SBUF).
- The graph-conv weight W is folded into the table (table_l = h_l @ W_{l+1}),
  so aggregation is identity-stationary accumulating matmuls straight into
  the pre-LN activation; conv bias enters via a rank-1 ones x bias matmul.
- LayerNorm: bn_stats/bn_aggr + ScalarE normalize; the per-feature affine
  (+ReLU) runs on the transposed tile so gamma/beta are per-partition scalars;
  the transposed result feeds the next table row + JK partial via one matmul
  against [W_next | Wo_l].
- AllGather of 256B-padded table rows between layers; JK partials parked in
  DRAM; final LN fused into the stage-2 block loop.
"""

import numpy as np

N = 50000
E = 800000
D = 96
ELEM = 128                   # fp16 elems per table row (256B, dma_gather min)
DOUT = 64
NCORES = 8
CHUNK = N // NCORES          # 6250
P = 128
NB = (CHUNK + P - 1) // P    # 49 blocks (last has 106 rows)
NLO = 5 * CHUNK              # 31250 rows in table-lo (cores 0-4)
EPS = 1e-5
TBUF = 32                    # max slot-columns per gather buffer
QBYTES = 160 * 1024          # SBUF budget for the replicated Q table
F16 = np.float16


def _wrap_idx(flat):
    """flat [n] -> wrapped [128, n//16] int16 (16-partition wrap, 8x replic)."""
    n = len(flat)
    w16 = np.zeros((16, n // 16), dtype=np.int16)
    j = np.arange(n)
    w16[j % 16, j // 16] = flat
    return np.tile(w16, (8, 1))


def _plan(lo_cnt, hi_cnt, core_of, blk_of):
    """Shared (cross-core) per-block slot bounds + gather batches + spill."""
    klos, khis = [], []
    for b in range(NB):
        sel = blk_of == b
        kl = max(int(lo_cnt[sel].max()), 1)
        kh = max(int(hi_cnt[sel].max()), 1)
        kl += kl & 1
        kh += kh & 1
        klos.append(kl)
        khis.append(kh)
    koffs = [0]
    for b in range(NB):
        koffs.append(koffs[-1] + klos[b] + khis[b])
    sumk = koffs[-1]

    batches = []
    cur, clo, chi = [], 0, 0
    for b in range(NB):
        if cur and (clo + klos[b] > TBUF or chi + khis[b] > TBUF):
            batches.append(cur)
            cur, clo, chi = [], 0, 0
        cur.append(b)
        clo += klos[b]
        chi += khis[b]
    batches.append(cur)

    # spill prefix blocks so the resident Q table fits QBYTES
    qmax_cols = QBYTES // (D * 2)
    sp = 0
    while sumk - koffs[sp] > qmax_cols:
        sp += 1
    return klos, khis, koffs, batches, sp


def _host_preprocess(x, src, dst, edge_weight, W0):
    src = np.asarray(src).astype(np.int64)
    dst = np.asarray(dst).astype(np.int64)
    ew = np.asarray(edge_weight).astype(np.float32)
    x = np.asarray(x).astype(np.float32)

    deg_out = np.maximum(np.bincount(src, minlength=N), 1).astype(np.float32)
    deg_in_raw = np.bincount(dst, minlength=N)
    deg_in = np.maximum(deg_in_raw, 1).astype(np.float32)
    q = ew / (np.sqrt(deg_out[src]) * np.sqrt(deg_in[dst]))

    order0 = np.argsort(-deg_in_raw, kind="stable")
    rank = np.empty(N, dtype=np.int64)
    rank[order0] = np.arange(N)
    core_of = (rank % NCORES).astype(np.int32)

    lo_edge = core_of[src] <= 4
    lo_cnt = np.bincount(dst[lo_edge], minlength=N).astype(np.int32)
    hi_cnt = (deg_in_raw - lo_cnt).astype(np.int32)

    blk_of = np.empty(N, dtype=np.int32)
    row_of = np.empty(N, dtype=np.int32)
    for c in range(NCORES):
        nodes = np.flatnonzero(core_of == c)
        o = nodes[np.lexsort((-hi_cnt[nodes], -lo_cnt[nodes]))]
        j = np.arange(CHUNK)
        blk_of[o] = j // P
        row_of[o] = j % P
    pos = core_of.astype(np.int64) * CHUNK + blk_of * P + row_of

    klos, khis, koffs, batches, sp = _plan(lo_cnt, hi_cnt, core_of, blk_of)
    sumk = koffs[-1]
    klos_a = np.asarray(klos)
    koffs_a = np.asarray(koffs[:-1])

    # per-edge slot assignment
    half = (~lo_edge).astype(np.int64)               # 0 = lo, 1 = hi
    key = dst * 2 + half
    es = np.argsort(key, kind="stable")
    ks = key[es]
    first = np.r_[True, ks[1:] != ks[:-1]]
    grp_start_idx = np.flatnonzero(first)
    grp_id = np.cumsum(first) - 1
    t_in = np.arange(E) - grp_start_idx[grp_id]

    vd = dst[es]
    hb = half[es]
    bb = blk_of[vd]
    col = koffs_a[bb] + np.where(hb == 1, klos_a[bb], 0) + t_in

    sidx = np.zeros((NCORES, P, sumk), dtype=np.int16)
    qv = np.zeros((NCORES, P, sumk), dtype=np.float32)
    val = np.where(hb == 1, pos[src[es]] - NLO, pos[src[es]]).astype(np.int16)
    sidx[core_of[vd], row_of[vd], col] = val
    qv[core_of[vd], row_of[vd], col] = q[es]

    # wrapped idx arrays, concatenated per (batch, half) op
    op_cols = []      # (col_start, ncols, is_hi) per op in program order
    for batch in batches:
        lo_cols, hi_cols = [], []
        for b in batch:
            lo_cols.append((koffs[b], klos[b]))
            hi_cols.append((koffs[b] + klos[b], khis[b]))
        op_cols.append(("lo", lo_cols))
        op_cols.append(("hi", hi_cols))

    idx_parts = [[] for _ in range(NCORES)]
    op_offs = []      # (idx_col_offset, ncols) per op
    off = 0
    for _hname, cols in op_cols:
        ncols = sum(n for (_s, n) in cols)
        op_offs.append((off, ncols))
        for c in range(NCORES):
            flat = np.concatenate(
                [sidx[c, :, s : s + n].T.ravel() for (s, n) in cols]
            )
            idx_parts[c].append(_wrap_idx(flat))
        off += ncols * 8
    idx_all = [
        np.ascontiguousarray(np.concatenate(parts, axis=1))
        for parts in idx_parts
    ]

    qwide = [
        np.ascontiguousarray(
            np.repeat(qv[c][:, koffs[sp] :, None], D, axis=2)
        ).astype(F16)
        for c in range(NCORES)
    ]
    qcomp = [np.ascontiguousarray(qv[c]) for c in range(NCORES)]

    t0 = x @ np.asarray(W0, np.float32)
    xp = np.zeros((N, ELEM), dtype=F16)
    xp[pos, :D] = t0.astype(F16)

    per_core = [
        {"sidx": idx_all[c], "qw": qwide[c], "qc": qcomp[c]}
        for c in range(NCORES)
    ]
    return per_core, xp, pos, klos, khis, koffs, batches, sp, op_offs


def _build_bass(klos, khis, koffs, batches, sp, op_offs):
    import concourse.bacc as bacc
    import concourse.mybir as mybir
    import concourse.tile as tile
    from concourse.masks import make_identity
    from contextlib import ExitStack

    dt = mybir.dt
    Alu = mybir.AluOpType
    Act = mybir.ActivationFunctionType
    sumk = koffs[-1]
    qcols = sumk - koffs[sp]

    nc = bacc.Bacc(
        "TRN2", target_bir_lowering=False, debug=False, num_devices=NCORES
    )

    h0 = nc.dram_tensor("h0", [N, ELEM], dt.float16, kind="ExternalInput")
    sidx = nc.dram_tensor("sidx", [P, sumk * 8], dt.int16, kind="ExternalInput")
    qw = nc.dram_tensor("qw", [P, qcols, D], dt.float16, kind="ExternalInput")
    qc = nc.dram_tensor("qc", [P, sumk], dt.float32, kind="ExternalInput")
    wexts = [
        nc.dram_tensor(f"wext{l}", [D, 160 if l < 2 else DOUT],
                       dt.float16, kind="ExternalInput")
        for l in range(3)
    ]
    bbs = [
        nc.dram_tensor(f"bb{l}", [1, D], dt.float16, kind="ExternalInput")
        for l in range(3)
    ]
    brow0 = nc.dram_tensor("brow0", [1, 160], dt.float16, kind="ExternalInput")
    gcols = [
        nc.dram_tensor(f"gc{l}", [D, 1], dt.float32, kind="ExternalInput")
        for l in range(3)
    ]
    becols = [
        nc.dram_tensor(f"bec{l}", [D, 1], dt.float32, kind="ExternalInput")
        for l in range(3)
    ]
    gob = nc.dram_tensor("gob", [P, DOUT], dt.float32, kind="ExternalInput")
    beob = nc.dram_tensor("beob", [P, DOUT], dt.float32, kind="ExternalInput")
    out = nc.dram_tensor("out", [CHUNK, DOUT], dt.float32, kind="ExternalOutput")

    with tile.TileContext(nc) as tc, ExitStack() as ctx:
        cpool = ctx.enter_context(tc.tile_pool(name="const", bufs=1))
        wpool = ctx.enter_context(tc.tile_pool(name="work", bufs=3))
        gpool = ctx.enter_context(tc.tile_pool(name="gath", bufs=2))
        ipool = ctx.enter_context(tc.tile_pool(name="idx", bufs=2))
        ppool = ctx.enter_context(tc.tile_pool(name="ps", bufs=2, space="PSUM"))
        dram = ctx.enter_context(tc.tile_pool(name="dram", bufs=1, space="DRAM"))

        h_loc = [
            dram.tile([CHUNK, ELEM], dt.float16, name=f"hloc{l}")
            for l in range(2)
        ]
        h_full = [
            dram.tile([N, ELEM], dt.float16, addr_space="Shared",
                      name=f"hfull{l}")
            for l in range(2)
        ]
        r_dram = [
            dram.tile([CHUNK, DOUT], dt.float32, name=f"r{l}") for l in range(2)
        ]

        id128h = cpool.tile([P, P], dt.float16, name="id128h")
        make_identity(nc, id128h[:])
        ones_row = cpool.tile([1, P], dt.float16, name="ones_row")
        nc.vector.memset(ones_row[:], 1.0)
        eps1 = cpool.tile([P, 1], dt.float32, name="eps1")
        nc.vector.memset(eps1[:], EPS)

        q3 = cpool.tile([P, qcols, D], dt.float16, name="q3")
        nc.sync.dma_start(out=q3[:], in_=qw[:])
        qc_sb = cpool.tile([P, sumk], dt.float32, name="qc_sb")
        nc.sync.dma_start(out=qc_sb[:], in_=qc[:])
        wext_sb, bb_sb, g_sb, be_sb = [], [], [], []
        for l in range(3):
            wc = 160 if l < 2 else DOUT
            t = cpool.tile([D, wc], dt.float16, name=f"wext{l}")
            nc.sync.dma_start(out=t[:], in_=wexts[l][:])
            wext_sb.append(t)
            t = cpool.tile([1, D], dt.float16, name=f"bb{l}")
            nc.sync.dma_start(out=t[:], in_=bbs[l][:])
            bb_sb.append(t)
            t = cpool.tile([D, 1], dt.float32, name=f"gc{l}")
            nc.sync.dma_start(out=t[:], in_=gcols[l][:])
            g_sb.append(t)
            t = cpool.tile([D, 1], dt.float32, name=f"bec{l}")
            nc.sync.dma_start(out=t[:], in_=becols[l][:])
            be_sb.append(t)
        brow0_sb = cpool.tile([1, 160], dt.float16, name="brow0")
        nc.sync.dma_start(out=brow0_sb[:], in_=brow0[:])
        gob_sb = cpool.tile([P, DOUT], dt.float32, name="gob")
        nc.sync.dma_start(out=gob_sb[:], in_=gob[:])
        beob_sb = cpool.tile([P, DOUT], dt.float32, name="beob")
        nc.sync.dma_start(out=beob_sb[:], in_=beob[:])

        for l in range(3):
            table = h0 if l == 0 else h_full[l - 1]
            tlo = table[0:NLO]
            thi = table[NLO:N]
            for bi, batch in enumerate(batches):
                (olo_off, olo_n) = op_offs[2 * bi]
                (ohi_off, ohi_n) = op_offs[2 * bi + 1]
                ilo = ipool.tile([P, TBUF * 8], dt.int16, tag="ilo")
                nc.sync.dma_start(
                    out=ilo[:, : olo_n * 8],
                    in_=sidx[:, olo_off : olo_off + olo_n * 8],
                )
                gwlo = gpool.tile([P, TBUF, ELEM], dt.float16, tag="glo")
                nc.gpsimd.dma_gather(
                    gwlo[:, :olo_n, :], tlo, ilo[:, : olo_n * 8],
                    olo_n * P, olo_n * P, ELEM, single_packet=False,
                )
                ihi = ipool.tile([P, TBUF * 8], dt.int16, tag="ihi")
                nc.sync.dma_start(
                    out=ihi[:, : ohi_n * 8],
                    in_=sidx[:, ohi_off : ohi_off + ohi_n * 8],
                )
                gwhi = gpool.tile([P, TBUF, ELEM], dt.float16, tag="ghi")
                nc.gpsimd.dma_gather(
                    gwhi[:, :ohi_n, :], thi, ihi[:, : ohi_n * 8],
                    ohi_n * P, ohi_n * P, ELEM, single_packet=False,
                )

                lo_off = hi_off = 0
                for b in batch:
                    klo, khi = klos[b], khis[b]
                    vr = P if b < NB - 1 else CHUNK - P * (NB - 1)
                    rows = slice(b * P, b * P + vr)
                    glo = gwlo[:, lo_off : lo_off + klo, :D]
                    ghi = gwhi[:, hi_off : hi_off + khi, :D]
                    if b < sp:
                        for t in range(klo):
                            g = koffs[b] + t
                            nc.vector.tensor_scalar(
                                out=gwlo[:, lo_off + t, :D],
                                in0=gwlo[:, lo_off + t, :D],
                                scalar1=qc_sb[:, g : g + 1], scalar2=None,
                                op0=Alu.mult,
                            )
                        for t in range(khi):
                            g = koffs[b] + klo + t
                            nc.vector.tensor_scalar(
                                out=gwhi[:, hi_off + t, :D],
                                in0=gwhi[:, hi_off + t, :D],
                                scalar1=qc_sb[:, g : g + 1], scalar2=None,
                                op0=Alu.mult,
                            )
                    else:
                        qo = koffs[b] - koffs[sp]
                        nc.vector.tensor_tensor(
                            out=glo, in0=glo,
                            in1=q3[:, qo : qo + klo, :], op=Alu.mult,
                        )
                        nc.vector.tensor_tensor(
                            out=ghi, in0=ghi,
                            in1=q3[:, qo + klo : qo + klo + khi, :],
                            op=Alu.mult,
                        )

                    c_ps = ppool.tile([P, D], dt.float32, tag="c", space="PSUM")
                    nc.tensor.matmul(
                        out=c_ps[:], lhsT=ones_row[:], rhs=bb_sb[l][:],
                        start=True, stop=False,
                    )
                    for t in range(klo):
                        nc.tensor.matmul(
                            out=c_ps[:], lhsT=id128h[:],
                            rhs=gwlo[:, lo_off + t, :D],
                            start=False, stop=False,
                        )
                    for t in range(khi):
                        nc.tensor.matmul(
                            out=c_ps[:], lhsT=id128h[:],
                            rhs=gwhi[:, hi_off + t, :D],
                            start=False, stop=(t == khi - 1),
                        )
                    lo_off += klo
                    hi_off += khi

                    stats = wpool.tile([P, 6], dt.float32, tag="stats")
                    nc.vector.bn_stats(out=stats[:], in_=c_ps[:])
                    mv = wpool.tile([P, 2], dt.float32, tag="mv")
                    nc.vector.bn_aggr(out=mv[:], in_=stats[:])
                    std = wpool.tile([P, 1], dt.float32, tag="std")
                    nc.scalar.activation(
                        out=std[:], in_=mv[:, 1:2], func=Act.Sqrt,
                        bias=eps1[:, :1],
                    )
                    rstd = wpool.tile([P, 1], dt.float32, tag="rstd")
                    nc.vector.reciprocal(out=rstd[:], in_=std[:])
                    nmr = wpool.tile([P, 1], dt.float32, tag="nmr")
                    nc.vector.tensor_scalar(
                        out=nmr[:], in0=mv[:, 0:1], scalar1=rstd[:, :1],
                        scalar2=-1.0, op0=Alu.mult, op1=Alu.mult,
                    )
                    yhat = wpool.tile([P, D], dt.float16, tag="yhat")
                    nc.scalar.activation(
                        out=yhat[:], in_=c_ps[:], func=Act.Identity,
                        scale=rstd[:, :1], bias=nmr[:, :1],
                    )
                    yT_ps = ppool.tile([D, P], dt.float16, tag="yT",
                                       space="PSUM")
                    nc.tensor.transpose(
                        out=yT_ps[:], in_=yhat[:], identity=id128h[:]
                    )
                    hT = wpool.tile([D, P], dt.float16, tag="hT")
                    nc.scalar.activation(
                        out=hT[:], in_=yT_ps[:],
                        func=Act.Relu if l < 2 else Act.Identity,
                        scale=g_sb[l][:, :1], bias=be_sb[l][:, :1],
                    )
                    wc = 160 if l < 2 else DOUT
                    ext_ps = ppool.tile([P, wc], dt.float32, tag="ext",
                                        space="PSUM")
                    if l == 0:
                        nc.tensor.matmul(
                            out=ext_ps[:], lhsT=ones_row[:], rhs=brow0_sb[:],
                            start=True, stop=False,
                        )
                    nc.tensor.matmul(
                        out=ext_ps[:], lhsT=hT[:], rhs=wext_sb[l][:],
                        start=(l != 0), stop=True,
                    )
                    if l < 2:
                        t16 = wpool.tile([P, ELEM], dt.float16, tag="t16")
                        nc.scalar.activation(
                            out=t16[:, :D], in_=ext_ps[:, :D], func=Act.Copy
                        )
                        nc.sync.dma_start(out=h_loc[l][rows], in_=t16[:vr])
                        rsb = wpool.tile([P, DOUT], dt.float32, tag="rsb")
                        nc.scalar.activation(
                            out=rsb[:], in_=ext_ps[:, D : D + DOUT],
                            func=Act.Copy,
                        )
                        nc.sync.dma_start(out=r_dram[l][rows], in_=rsb[:vr])
                    else:
                        r0sb = wpool.tile([P, DOUT], dt.float32, tag="r0sb")
                        nc.sync.dma_start(out=r0sb[:vr], in_=r_dram[0][rows])
                        r1sb = wpool.tile([P, DOUT], dt.float32, tag="r1sb")
                        nc.sync.dma_start(out=r1sb[:vr], in_=r_dram[1][rows])
                        f01 = wpool.tile([P, DOUT], dt.float32, tag="f01")
                        nc.vector.tensor_tensor(
                            out=f01[:], in0=r0sb[:], in1=r1sb[:], op=Alu.add
                        )
                        fsb = wpool.tile([P, DOUT], dt.float32, tag="fsb")
                        nc.vector.tensor_tensor(
                            out=fsb[:], in0=f01[:], in1=ext_ps[:], op=Alu.add
                        )
                        statf = wpool.tile([P, 6], dt.float32, tag="statf")
                        nc.vector.bn_stats(out=statf[:], in_=fsb[:])
                        mvf = wpool.tile([P, 2], dt.float32, tag="mvf")
                        nc.vector.bn_aggr(out=mvf[:], in_=statf[:])
                        stdf = wpool.tile([P, 1], dt.float32, tag="stdf")
                        nc.scalar.activation(
                            out=stdf[:], in_=mvf[:, 1:2], func=Act.Sqrt,
                            bias=eps1[:, :1],
                        )
                        rstdf = wpool.tile([P, 1], dt.float32, tag="rstdf")
                        nc.vector.reciprocal(out=rstdf[:], in_=stdf[:])
                        yf = wpool.tile([P, DOUT], dt.float32, tag="yf")
                        nc.vector.tensor_scalar(
                            out=yf[:], in0=fsb[:], scalar1=mvf[:, 0:1],
                            scalar2=rstdf[:, :1], op0=Alu.subtract,
                            op1=Alu.mult,
                        )
                        yg = wpool.tile([P, DOUT], dt.float32, tag="yg")
                        nc.vector.tensor_tensor(
                            out=yg[:], in0=yf[:], in1=gob_sb[:], op=Alu.mult
                        )
                        yo = wpool.tile([P, DOUT], dt.float32, tag="yo")
                        nc.vector.tensor_tensor(
                            out=yo[:], in0=yg[:], in1=beob_sb[:], op=Alu.add
                        )
                        nc.sync.dma_start(out=out[rows], in_=yo[:vr])

            if l < 2:
                nc.gpsimd.collective_compute(
                    "AllGather",
                    Alu.bypass,
                    ins=[h_loc[l][:]],
                    outs=[h_full[l][:]],
                    replica_groups=[list(range(NCORES))],
                )

    nc.finalize()
    return nc


_CACHE = {}


def kernel(
    x, src, dst, edge_weight,
    W0, b0, g0, be0, W1, b1, g1, be1, W2, b2, g2, be2,
    Wo, bo, go, beo,
):
    from concourse import bass_utils

    (per_core, xp, pos, klos, khis, koffs, batches, sp,
     op_offs) = _host_preprocess(x, src, dst, edge_weight, W0)

    key = (tuple(klos), tuple(khis), sp)
    if key not in _CACHE:
        _CACHE[key] = _build_bass(klos, khis, koffs, batches, sp, op_offs)
    nc = _CACHE[key]

    W1a = np.asarray(W1, np.float32)
    W2a = np.asarray(W2, np.float32)
    Woa = np.asarray(Wo, np.float32)
    Wos = [Woa[0:D], Woa[D : 2 * D], Woa[2 * D : 3 * D]]

    wext_h = [
        np.ascontiguousarray(np.concatenate([W1a, Wos[0]], axis=1)).astype(F16),
        np.ascontiguousarray(np.concatenate([W2a, Wos[1]], axis=1)).astype(F16),
        np.ascontiguousarray(Wos[2]).astype(F16),
    ]
    brow0_h = np.zeros((1, 160), np.float32)
    brow0_h[0, D:] = np.asarray(bo, np.float32)

    common = {
        "h0": xp,
        "wext0": wext_h[0], "wext1": wext_h[1], "wext2": wext_h[2],
        "bb0": np.asarray(b0, np.float32).reshape(1, D).astype(F16),
        "bb1": np.asarray(b1, np.float32).reshape(1, D).astype(F16),
        "bb2": np.asarray(b2, np.float32).reshape(1, D).astype(F16),
        "brow0": brow0_h.astype(F16),
        "gc0": np.asarray(g0, np.float32).reshape(D, 1),
        "gc1": np.asarray(g1, np.float32).reshape(D, 1),
        "gc2": np.asarray(g2, np.float32).reshape(D, 1),
        "bec0": np.asarray(be0, np.float32).reshape(D, 1),
        "bec1": np.asarray(be1, np.float32).reshape(D, 1),
        "bec2": np.asarray(be2, np.float32).reshape(D, 1),
        "gob": np.ascontiguousarray(
            np.broadcast_to(np.asarray(go, np.float32).reshape(1, DOUT),
                            (P, DOUT))
        ),
        "beob": np.ascontiguousarray(
            np.broadcast_to(np.asarray(beo, np.float32).reshape(1, DOUT),
                            (P, DOUT))
        ),
    }
    in_maps = [dict(common, **per_core[c]) for c in range(NCORES)]

    import os

    res = bass_utils.run_bass_kernel_spmd(
        nc,
        in_maps,
        core_ids=list(range(NCORES)),
        trace=bool(os.environ.get("BASS_TRACE")),
    )
    y_perm = np.concatenate([r["out"] for r in res.results], axis=0)
    if res.exec_time_ns is not None:
        kernel.last_exec_time_ns = res.exec_time_ns
    kernel.last_results = res
    return y_perm[pos].astype(np.float32)


# revision 6
# speedup vs baseline: 1.3856x; 1.3856x over previous
"""JKNet (3x GraphConv+LN+ReLU, JK-concat, Linear, LN) on 8 Trainium2 cores.

v4 strategy (descriptor-rate-bound design):
- The SWDGE Q7 emits gather descriptors at ~8ns/row — the hard wall for any
  random-gather GNN layer on this part. So the design minimizes gathered
  slots: edges are packed DENSELY per (dst-block, table-half) with no
  per-node rectangles; scatter+scale into the pre-LN activation happens on
  the TensorEngine via host-built S matrices (S[slot, v] = q_e, folded
  degree norms + edge weight), streamed from DRAM as big sequential DMAs.
- Nodes are dealt round-robin (by global in-degree rank) to 8 cores, and
  within a core ordered by (-lo_cnt, -hi_cnt) into 49 blocks of 128 so that
  per-(core, block) slot counts are nearly equal; cross-core maxima give an
  SPMD-uniform program (pad slots gather row 0 with a zero S-row).
- lo/hi halves: whether the source's table row is < 31250 (cores 0-4) —
  both halves fit the Ant dma_gather's int16 indices. One dma_gather per
  (section of blocks, half) over thousands of indices.
- The graph-conv weight W is folded into the table (table_l = h_l @ W_{l+1});
  conv bias enters via a rank-1 ones x bias matmul. Gathered tiles that span
  a block boundary get two S tiles (one per block).
- LayerNorm: bn_stats/bn_aggr + ScalarE normalize; per-feature affine(+ReLU)
  on the transposed tile; one matmul against [W_next | Wo_l] produces the
  next table row and the JK partial. AllGather of 256B-padded table rows
  between layers; JK partials parked in DRAM; final LN fused into stage 2.
"""

import numpy as np

N = 50000
E = 800000
D = 96
ELEM = 128                   # fp16 elems per table row (256B, dma_gather min)
DOUT = 64
NCORES = 8
CHUNK = N // NCORES          # 6250
P = 128
NB = (CHUNK + P - 1) // P    # 49 blocks (last has 106 rows)
NLO = 5 * CHUNK              # 31250 rows in table-lo (cores 0-4)
EPS = 1e-5
SECBLK = 3                   # blocks per gather section
F16 = np.float16


def _host_preprocess(x, src, dst, edge_weight, W0):
    src = np.asarray(src).astype(np.int64)
    dst = np.asarray(dst).astype(np.int64)
    ew = np.asarray(edge_weight).astype(np.float32)
    x = np.asarray(x).astype(np.float32)

    deg_out = np.maximum(np.bincount(src, minlength=N), 1).astype(np.float32)
    deg_in_raw = np.bincount(dst, minlength=N)
    deg_in = np.maximum(deg_in_raw, 1).astype(np.float32)
    q = ew / (np.sqrt(deg_out[src]) * np.sqrt(deg_in[dst]))

    order0 = np.argsort(-deg_in_raw, kind="stable")
    rank = np.empty(N, dtype=np.int64)
    rank[order0] = np.arange(N)
    core_of = (rank % NCORES).astype(np.int32)

    lo_edge = core_of[src] <= 4
    lo_cnt = np.bincount(dst[lo_edge], minlength=N).astype(np.int32)
    hi_cnt = (deg_in_raw - lo_cnt).astype(np.int32)

    blk_of = np.empty(N, dtype=np.int32)
    row_of = np.empty(N, dtype=np.int32)
    for c in range(NCORES):
        nodes = np.flatnonzero(core_of == c)
        o = nodes[np.lexsort((-hi_cnt[nodes], -lo_cnt[nodes]))]
        j = np.arange(CHUNK)
        blk_of[o] = j // P
        row_of[o] = j % P
    pos = core_of.astype(np.int64) * CHUNK + blk_of * P + row_of

    # per-(core, block, half) edge counts; slot counts = cross-core max
    ebc = blk_of[dst]
    ecr = core_of[dst]
    half = (~lo_edge).astype(np.int64)
    cnt = np.zeros((NCORES, NB, 2), dtype=np.int64)
    np.add.at(cnt, (ecr, ebc, half), 1)
    cmax = cnt.max(axis=0)                    # [NB, 2] shared slot counts

    sections = [
        list(range(s, min(s + SECBLK, NB))) for s in range(0, NB, SECBLK)
    ]

    # schedule: per section, per half: op slot count (padded to 128),
    # per-block slot offsets; tile list (block, gw_tile, s_tile).
    op_plan = []
    s_tiles = [0, 0]
    idx_cols = [0, 0]
    for sec in sections:
        info = {}
        for h in (0, 1):
            offs = {}
            o = 0
            for b in sec:
                offs[b] = o
                o += int(cmax[b, h])
            nslots = -(-o // P) * P
            ntiles = nslots // P
            mm = []
            st = s_tiles[h]
            for ti in range(ntiles):
                t0, t1 = ti * P, (ti + 1) * P
                for b in sec:
                    b0, b1 = offs[b], offs[b] + int(cmax[b, h])
                    if b0 < t1 and t0 < b1:
                        mm.append((b, ti, st))
                        st += 1
            info[h] = dict(
                offs=offs, nslots=nslots, ntiles=ntiles, mm=mm,
                s_base=s_tiles[h], idx_off=idx_cols[h],
            )
            s_tiles[h] = st
            idx_cols[h] += nslots // 16
        op_plan.append(info)

    # per-edge slot index within its (core, block, half)
    key = (ecr.astype(np.int64) * NB + ebc) * 2 + half
    es = np.argsort(key, kind="stable")
    ks = key[es]
    first = np.r_[True, ks[1:] != ks[:-1]]
    grp_start_idx = np.flatnonzero(first)
    grp_id = np.cumsum(first) - 1
    t_in = np.arange(E) - grp_start_idx[grp_id]

    slot_off_in_op = np.zeros((NB, 2), dtype=np.int64)
    op_idx_off = np.zeros((NB, 2), dtype=np.int64)
    for si, sec in enumerate(sections):
        for h in (0, 1):
            info = op_plan[si][h]
            for b in sec:
                slot_off_in_op[b, h] = info["offs"][b]
                op_idx_off[b, h] = info["idx_off"]

    vd = dst[es]
    hb = half[es]
    bb_ = ebc[es]
    cc = ecr[es]
    slot_in_op = slot_off_in_op[bb_, hb] + t_in
    val = np.where(hb == 1, pos[src[es]] - NLO, pos[src[es]]).astype(np.int16)

    # idx arrays: within an op, idx j -> wrapped[16g + j%16, idx_off + j//16]
    idxw = [np.zeros((NCORES, P, idx_cols[h]), dtype=np.int16) for h in (0, 1)]
    for h in (0, 1):
        selh = hb == h
        j = slot_in_op[selh]
        c = cc[selh]
        colw = op_idx_off[bb_[selh], h] + j // 16
        roww = (j % 16).astype(np.int64)
        v = val[selh]
        for g in range(8):
            idxw[h][c, g * 16 + roww, colw] = v

    # S arrays: [core][half] -> [P(slot%128), s_tiles, P(v)] fp16
    s_tile_of = [dict(), dict()]
    for si, sec in enumerate(sections):
        for h in (0, 1):
            for (b, ti, sti) in op_plan[si][h]["mm"]:
                s_tile_of[h][(b, ti)] = sti
    s_arr = [np.zeros((NCORES, P, s_tiles[h], P), dtype=F16) for h in (0, 1)]
    qes = q[es].astype(F16)
    vrow = row_of[vd]
    for h in (0, 1):
        selh = hb == h
        j = slot_in_op[selh]
        b = bb_[selh]
        c = cc[selh]
        ti = j // P
        sl = j % P
        sti = np.fromiter(
            (s_tile_of[h][(bi, tii)] for bi, tii in zip(b, ti)),
            dtype=np.int64, count=len(b),
        )
        s_arr[h][c, sl, sti, vrow[selh]] = qes[selh]

    t0 = x @ np.asarray(W0, np.float32)
    xp = np.zeros((N, ELEM), dtype=F16)
    xp[pos, :D] = t0.astype(F16)

    per_core = [
        {
            "idxlo": np.ascontiguousarray(idxw[0][c]),
            "idxhi": np.ascontiguousarray(idxw[1][c]),
            "slo": np.ascontiguousarray(s_arr[0][c]),
            "shi": np.ascontiguousarray(s_arr[1][c]),
        }
        for c in range(NCORES)
    ]
    return per_core, xp, pos, sections, op_plan, s_tiles, idx_cols


def _build_bass(sections, op_plan, s_tiles, idx_cols):
    import concourse.bacc as bacc
    import concourse.mybir as mybir
    import concourse.tile as tile
    from concourse.masks import make_identity
    from contextlib import ExitStack

    dt = mybir.dt
    Alu = mybir.AluOpType
    Act = mybir.ActivationFunctionType

    max_tiles = [
        max(op_plan[si][h]["ntiles"] for si in range(len(sections)))
        for h in (0, 1)
    ]
    max_stiles = [
        max(len(op_plan[si][h]["mm"]) for si in range(len(sections)))
        for h in (0, 1)
    ]

    nc = bacc.Bacc(
        "TRN2", target_bir_lowering=False, debug=False, num_devices=NCORES
    )

    h0 = nc.dram_tensor("h0", [N, ELEM], dt.float16, kind="ExternalInput")
    idxlo = nc.dram_tensor("idxlo", [P, idx_cols[0]], dt.int16,
                           kind="ExternalInput")
    idxhi = nc.dram_tensor("idxhi", [P, idx_cols[1]], dt.int16,
                           kind="ExternalInput")
    slo = nc.dram_tensor("slo", [P, s_tiles[0], P], dt.float16,
                         kind="ExternalInput")
    shi = nc.dram_tensor("shi", [P, s_tiles[1], P], dt.float16,
                         kind="ExternalInput")
    wexts = [
        nc.dram_tensor(f"wext{l}", [D, 160 if l < 2 else DOUT],
                       dt.float16, kind="ExternalInput")
        for l in range(3)
    ]
    bbs = [
        nc.dram_tensor(f"bb{l}", [1, D], dt.float16, kind="ExternalInput")
        for l in range(3)
    ]
    brow0 = nc.dram_tensor("brow0", [1, 160], dt.float16, kind="ExternalInput")
    gcols = [
        nc.dram_tensor(f"gc{l}", [D, 1], dt.float32, kind="ExternalInput")
        for l in range(3)
    ]
    becols = [
        nc.dram_tensor(f"bec{l}", [D, 1], dt.float32, kind="ExternalInput")
        for l in range(3)
    ]
    gob = nc.dram_tensor("gob", [P, DOUT], dt.float32, kind="ExternalInput")
    beob = nc.dram_tensor("beob", [P, DOUT], dt.float32, kind="ExternalInput")
    out = nc.dram_tensor("out", [CHUNK, DOUT], dt.float32, kind="ExternalOutput")

    with tile.TileContext(nc) as tc, ExitStack() as ctx:
        cpool = ctx.enter_context(tc.tile_pool(name="const", bufs=1))
        wpool = ctx.enter_context(tc.tile_pool(name="work", bufs=3))
        gpool = ctx.enter_context(tc.tile_pool(name="gath", bufs=2))
        spool = ctx.enter_context(tc.tile_pool(name="smat", bufs=2))
        ipool = ctx.enter_context(tc.tile_pool(name="idx", bufs=2))
        ppool = ctx.enter_context(tc.tile_pool(name="ps", bufs=2, space="PSUM"))
        dram = ctx.enter_context(tc.tile_pool(name="dram", bufs=1, space="DRAM"))

        h_loc = [
            dram.tile([CHUNK, ELEM], dt.float16, name=f"hloc{l}")
            for l in range(2)
        ]
        h_full = [
            dram.tile([N, ELEM], dt.float16, addr_space="Shared",
                      name=f"hfull{l}")
            for l in range(2)
        ]
        r_dram = [
            dram.tile([CHUNK, DOUT], dt.float32, name=f"r{l}") for l in range(2)
        ]

        id128h = cpool.tile([P, P], dt.float16, name="id128h")
        make_identity(nc, id128h[:])
        ones_row = cpool.tile([1, P], dt.float16, name="ones_row")
        nc.vector.memset(ones_row[:], 1.0)
        eps1 = cpool.tile([P, 1], dt.float32, name="eps1")
        nc.vector.memset(eps1[:], EPS)

        wext_sb, bb_sb, g_sb, be_sb = [], [], [], []
        for l in range(3):
            wc = 160 if l < 2 else DOUT
            t = cpool.tile([D, wc], dt.float16, name=f"wext{l}")
            nc.sync.dma_start(out=t[:], in_=wexts[l][:])
            wext_sb.append(t)
            t = cpool.tile([1, D], dt.float16, name=f"bb{l}")
            nc.sync.dma_start(out=t[:], in_=bbs[l][:])
            bb_sb.append(t)
            t = cpool.tile([D, 1], dt.float32, name=f"gc{l}")
            nc.sync.dma_start(out=t[:], in_=gcols[l][:])
            g_sb.append(t)
            t = cpool.tile([D, 1], dt.float32, name=f"bec{l}")
            nc.sync.dma_start(out=t[:], in_=becols[l][:])
            be_sb.append(t)
        brow0_sb = cpool.tile([1, 160], dt.float16, name="brow0")
        nc.sync.dma_start(out=brow0_sb[:], in_=brow0[:])
        gob_sb = cpool.tile([P, DOUT], dt.float32, name="gob")
        nc.sync.dma_start(out=gob_sb[:], in_=gob[:])
        beob_sb = cpool.tile([P, DOUT], dt.float32, name="beob")
        nc.sync.dma_start(out=beob_sb[:], in_=beob[:])

        # zero the gather pool buffers once (pad slots read stale data)
        for _rep in range(2):
            for h, tg in ((0, "glo"), (1, "ghi")):
                t = gpool.tile([P, max_tiles[h], ELEM], dt.float16, tag=tg)
                nc.vector.memset(t[:], 0.0)

        for l in range(3):
            table = h0 if l == 0 else h_full[l - 1]
            tabs = [table[0:NLO], table[NLO:N]]
            idxs = [idxlo, idxhi]
            smats = [slo, shi]
            for si, sec in enumerate(sections):
                gw = []
                ssb = []
                for h in (0, 1):
                    info = op_plan[si][h]
                    nt = info["ntiles"]
                    nidx = info["nslots"]
                    icol0 = info["idx_off"]
                    isb = ipool.tile([P, max_tiles[h] * 8], dt.int16,
                                     tag=f"i{h}")
                    nc.sync.dma_start(
                        out=isb[:, : nidx // 16],
                        in_=idxs[h][:, icol0 : icol0 + nidx // 16],
                    )
                    g = gpool.tile([P, max_tiles[h], ELEM], dt.float16,
                                   tag="glo" if h == 0 else "ghi")
                    nc.gpsimd.dma_gather(
                        g[:, :nt, :], tabs[h], isb[:, : nidx // 16],
                        nidx, nidx, ELEM, single_packet=False,
                    )
                    gw.append(g)
                    nst = len(info["mm"])
                    ss = spool.tile([P, max_stiles[h], P], dt.float16,
                                    tag=f"s{h}")
                    if nst:
                        s0 = info["mm"][0][2]
                        nc.sync.dma_start(
                            out=ss[:, :nst, :],
                            in_=smats[h][:, s0 : s0 + nst, :],
                        )
                    ssb.append(ss)

                for b in sec:
                    vr = P if b < NB - 1 else CHUNK - P * (NB - 1)
                    rows = slice(b * P, b * P + vr)
                    mms = []
                    for h in (0, 1):
                        info = op_plan[si][h]
                        s0 = info["mm"][0][2] if info["mm"] else 0
                        for (bi, ti, sti) in info["mm"]:
                            if bi == b:
                                mms.append((h, ti, sti - s0))
                    c_ps = ppool.tile([P, D], dt.float32, tag="c", space="PSUM")
                    nc.tensor.matmul(
                        out=c_ps[:], lhsT=ones_row[:], rhs=bb_sb[l][:],
                        start=True, stop=False,
                    )
                    for mi, (h, ti, sk) in enumerate(mms):
                        nc.tensor.matmul(
                            out=c_ps[:],
                            lhsT=ssb[h][:, sk, :],
                            rhs=gw[h][:, ti, :D],
                            start=False, stop=(mi == len(mms) - 1),
                        )

                    stats = wpool.tile([P, 6], dt.float32, tag="stats")
                    nc.vector.bn_stats(out=stats[:], in_=c_ps[:])
                    mv = wpool.tile([P, 2], dt.float32, tag="mv")
                    nc.vector.bn_aggr(out=mv[:], in_=stats[:])
                    std = wpool.tile([P, 1], dt.float32, tag="std")
                    nc.scalar.activation(
                        out=std[:], in_=mv[:, 1:2], func=Act.Sqrt,
                        bias=eps1[:, :1],
                    )
                    rstd = wpool.tile([P, 1], dt.float32, tag="rstd")
                    nc.vector.reciprocal(out=rstd[:], in_=std[:])
                    nmr = wpool.tile([P, 1], dt.float32, tag="nmr")
                    nc.vector.tensor_scalar(
                        out=nmr[:], in0=mv[:, 0:1], scalar1=rstd[:, :1],
                        scalar2=-1.0, op0=Alu.mult, op1=Alu.mult,
                    )
                    yhat = wpool.tile([P, D], dt.float16, tag="yhat")
                    nc.scalar.activation(
                        out=yhat[:], in_=c_ps[:], func=Act.Identity,
                        scale=rstd[:, :1], bias=nmr[:, :1],
                    )
                    yT_ps = ppool.tile([D, P], dt.float16, tag="yT",
                                       space="PSUM")
                    nc.tensor.transpose(
                        out=yT_ps[:], in_=yhat[:], identity=id128h[:]
                    )
                    hT = wpool.tile([D, P], dt.float16, tag="hT")
                    nc.scalar.activation(
                        out=hT[:], in_=yT_ps[:],
                        func=Act.Relu if l < 2 else Act.Identity,
                        scale=g_sb[l][:, :1], bias=be_sb[l][:, :1],
                    )
                    wc = 160 if l < 2 else DOUT
                    ext_ps = ppool.tile([P, wc], dt.float32, tag="ext",
                                        space="PSUM")
                    if l == 0:
                        nc.tensor.matmul(
                            out=ext_ps[:], lhsT=ones_row[:], rhs=brow0_sb[:],
                            start=True, stop=False,
                        )
                    nc.tensor.matmul(
                        out=ext_ps[:], lhsT=hT[:], rhs=wext_sb[l][:],
                        start=(l != 0), stop=True,
                    )
                    if l < 2:
                        t16 = wpool.tile([P, ELEM], dt.float16, tag="t16")
                        nc.scalar.activation(
                            out=t16[:, :D], in_=ext_ps[:, :D], func=Act.Copy
                        )
                        nc.sync.dma_start(out=h_loc[l][rows], in_=t16[:vr])
                        rsb = wpool.tile([P, DOUT], dt.float32, tag="rsb")
                        nc.scalar.activation(
                            out=rsb[:], in_=ext_ps[:, D : D + DOUT],
                            func=Act.Copy,
                        )
                        nc.sync.dma_start(out=r_dram[l][rows], in_=rsb[:vr])
                    else:
                        r0sb = wpool.tile([P, DOUT], dt.float32, tag="r0sb")
                        nc.sync.dma_start(out=r0sb[:vr], in_=r_dram[0][rows])
                        r1sb = wpool.tile([P, DOUT], dt.float32, tag="r1sb")
                        nc.sync.dma_start(out=r1sb[:vr], in_=r_dram[1][rows])
                        f01 = wpool.tile([P, DOUT], dt.float32, tag="f01")
                        nc.vector.tensor_tensor(
                            out=f01[:], in0=r0sb[:], in1=r1sb[:], op=Alu.add
                        )
                        fsb = wpool.tile([P, DOUT], dt.float32, tag="fsb")
                        nc.vector.tensor_tensor(
                            out=fsb[:], in0=f01[:], in1=ext_ps[:], op=Alu.add
                        )
                        statf = wpool.tile([P, 6], dt.float32, tag="statf")
                        nc.vector.bn_stats(out=statf[:], in_=fsb[:])
                        mvf = wpool.tile([P, 2], dt.float32, tag="mvf")
                        nc.vector.bn_aggr(out=mvf[:], in_=statf[:])
                        stdf = wpool.tile([P, 1], dt.float32, tag="stdf")
                        nc.scalar.activation(
                            out=stdf[:], in_=mvf[:, 1:2], func=Act.Sqrt,
                            bias=eps1[:, :1],
                        )
                        rstdf = wpool.tile([P, 1], dt.float32, tag="rstdf")
                        nc.vector.reciprocal(out=rstdf[:], in_=stdf[:])
                        yf = wpool.tile([P, DOUT], dt.float32, tag="yf")
                        nc.vector.tensor_scalar(
                            out=yf[:], in0=fsb[:], scalar1=mvf[:, 0:1],
                            scalar2=rstdf[:, :1], op0=Alu.subtract,
                            op1=Alu.mult,
                        )
                        yg = wpool.tile([P, DOUT], dt.float32, tag="yg")
                        nc.vector.tensor_tensor(
                            out=yg[:], in0=yf[:], in1=gob_sb[:], op=Alu.mult
                        )
                        yo = wpool.tile([P, DOUT], dt.float32, tag="yo")
                        nc.vector.tensor_tensor(
                            out=yo[:], in0=yg[:], in1=beob_sb[:], op=Alu.add
                        )
                        nc.sync.dma_start(out=out[rows], in_=yo[:vr])

            if l < 2:
                nc.gpsimd.collective_compute(
                    "AllGather",
                    Alu.bypass,
                    ins=[h_loc[l][:]],
                    outs=[h_full[l][:]],
                    replica_groups=[list(range(NCORES))],
                )

    nc.finalize()
    return nc


_CACHE = {}


def kernel(
    x, src, dst, edge_weight,
    W0, b0, g0, be0, W1, b1, g1, be1, W2, b2, g2, be2,
    Wo, bo, go, beo,
):
    from concourse import bass_utils

    (per_core, xp, pos, sections, op_plan, s_tiles,
     idx_cols) = _host_preprocess(x, src, dst, edge_weight, W0)

    key = (tuple(s_tiles), tuple(idx_cols))
    if key not in _CACHE:
        _CACHE[key] = _build_bass(sections, op_plan, s_tiles, idx_cols)
    nc = _CACHE[key]

    W1a = np.asarray(W1, np.float32)
    W2a = np.asarray(W2, np.float32)
    Woa = np.asarray(Wo, np.float32)
    Wos = [Woa[0:D], Woa[D : 2 * D], Woa[2 * D : 3 * D]]

    wext_h = [
        np.ascontiguousarray(np.concatenate([W1a, Wos[0]], axis=1)).astype(F16),
        np.ascontiguousarray(np.concatenate([W2a, Wos[1]], axis=1)).astype(F16),
        np.ascontiguousarray(Wos[2]).astype(F16),
    ]
    brow0_h = np.zeros((1, 160), np.float32)
    brow0_h[0, D:] = np.asarray(bo, np.float32)

    common = {
        "h0": xp,
        "wext0": wext_h[0], "wext1": wext_h[1], "wext2": wext_h[2],
        "bb0": np.asarray(b0, np.float32).reshape(1, D).astype(F16),
        "bb1": np.asarray(b1, np.float32).reshape(1, D).astype(F16),
        "bb2": np.asarray(b2, np.float32).reshape(1, D).astype(F16),
        "brow0": brow0_h.astype(F16),
        "gc0": np.asarray(g0, np.float32).reshape(D, 1),
        "gc1": np.asarray(g1, np.float32).reshape(D, 1),
        "gc2": np.asarray(g2, np.float32).reshape(D, 1),
        "bec0": np.asarray(be0, np.float32).reshape(D, 1),
        "bec1": np.asarray(be1, np.float32).reshape(D, 1),
        "bec2": np.asarray(be2, np.float32).reshape(D, 1),
        "gob": np.ascontiguousarray(
            np.broadcast_to(np.asarray(go, np.float32).reshape(1, DOUT),
                            (P, DOUT))
        ),
        "beob": np.ascontiguousarray(
            np.broadcast_to(np.asarray(beo, np.float32).reshape(1, DOUT),
                            (P, DOUT))
        ),
    }
    in_maps = [dict(common, **per_core[c]) for c in range(NCORES)]

    import os

    res = bass_utils.run_bass_kernel_spmd(
        nc,
        in_maps,
        core_ids=list(range(NCORES)),
        trace=bool(os.environ.get("BASS_TRACE")),
    )
    y_perm = np.concatenate([r["out"] for r in res.results], axis=0)
    if res.exec_time_ns is not None:
        kernel.last_exec_time_ns = res.exec_time_ns
    kernel.last_results = res
    return y_perm[pos].astype(np.float32)
